# revision 27
# baseline (speedup 1.0000x reference)
"""AttentionGRUDecoder Trainium2 kernel: 8-core batch-parallel (16 batch/core).

Strategy:
  - Shard B=128 across 8 cores (16 each); replicate weights. No collectives.
  - encoder_outputs resident in SBUF as bf16 (16 MB/core); streamed through PE
    each step via column-tiled (tile_position) matmuls, 4 concurrent streams.
  - dec_in-dependent matmuls (dec @ WaV.T, dec @ WcV.T) hoisted out of the
    recurrence, computed per 10-step chunk, overlapped with the loop.
  - Output projection (h @ Wo.T) batched after the loop over all 150 steps.
  - h kept in fp32 between steps; bf16 only feeds matmuls.
"""
import sys
sys.path.insert(0, '/opt/trn_rl_repo')
import numpy as np
import ml_dtypes

import os
B, T, H, A, V = 128, 150, 512, 512, 600
T = int(os.environ.get("KERNEL_T", T))
S = 2 * A            # 1024 attention positions
NC = 8               # cores
BL = B // NC         # 16 local batch
VP = 640             # V padded to 5*128
CH = min(8, T)       # steps per precompute chunk
NCH = (T + CH - 1) // CH   # chunks (last may be ragged)
TB = T * BL          # (t, b) columns

bf16 = ml_dtypes.bfloat16

_CACHE = {}


def _build():
    import concourse.bass as bass
    import concourse.mybir as mybir
    import concourse.tile as tile
    from concourse import bacc

    f32 = mybir.dt.float32
    bf = mybir.dt.bfloat16

    nc = bacc.Bacc("TRN2", target_bir_lowering=False, debug=False, num_devices=NC)
    dram = lambda n, s, d, k: nc.dram_tensor(n, s, d, kind=k).ap()

    # inputs (host-prepped layouts)
    E_ext = dram("e", [BL, 8, 128, A], bf, "ExternalInput")          # [b, st, sp, a]
    h0T_ext = dram("h0t", [128, 4, BL], bf, "ExternalInput")         # [kp, kt, b]
    h0_ext = dram("h0", [BL, H], f32, "ExternalInput")
    decT_ext = dram("dect", [5, 128, TB], bf, "ExternalInput")       # [vt, vp, (t,b)]
    waht_ext = dram("waht", [128, 4, 8, 128], bf, "ExternalInput")   # [kp, kt, st, ss]
    wavt_ext = dram("wavt", [5, 128, 8, 128], bf, "ExternalInput")   # [vt, vp, st, ss]
    wcat_ext = dram("wcat", [128, 4, 4, 128], bf, "ExternalInput")   # [ap, at, ot, os]
    wcvt_ext = dram("wcvt", [5, 128, 4, 128], bf, "ExternalInput")
    wihh_ext = dram("wihh", [128, 4, 2 * 3 * H], bf, "ExternalInput")  # [kp, kt, 3072]
    wot_ext = dram("wot", [128, 4, V], bf, "ExternalInput")          # [kp, kt, o]
    sel4_ext = dram("sel4", [128, 4], bf, "ExternalInput")
    ones_ext = dram("ones", [128, 2], bf, "ExternalInput")           # col 0: ones
    onesr_ext = dram("onesr", [1, 128], bf, "ExternalInput")
    id16_ext = dram("id16", [16, 16], bf, "ExternalInput")

    out_ext = dram("out", [BL, T, V], f32, "ExternalOutput")
    hf_ext = dram("hf", [BL, H], f32, "ExternalOutput")
    hstage = dram("hstage", [128, 4, TB], bf, "Internal")

    with tile.TileContext(nc) as tc:
        with tc.tile_pool(name="persist", bufs=1) as pp, \
             tc.tile_pool(name="wstream", bufs=2) as wsp, \
             tc.tile_pool(name="chunk", bufs=2) as cp, \
             tc.tile_pool(name="dpool", bufs=1) as dp, \
             tc.tile_pool(name="npool", bufs=1) as npo, \
             tc.tile_pool(name="step", bufs=2) as sp, \
             tc.tile_pool(name="tmp", bufs=2) as tp, \
             tc.tile_pool(name="pmix", bufs=2, space="PSUM") as pmix, \
             tc.tile_pool(name="patt", bufs=2, space="PSUM") as patt, \
             tc.tile_pool(name="prz", bufs=1, space="PSUM") as prz, \
             tc.tile_pool(name="pn", bufs=2, space="PSUM") as pn:

            # ---- persistent loads ----
            E_sb = pp.tile([128, BL, 8, A], bf)
            for b in range(BL):
                nc.sync.dma_start(
                    out=E_sb[:, b, :, :],
                    in_=E_ext[b].rearrange("st sp a -> sp st a"))
            waht = pp.tile([128, 4, 8, 128], bf)
            nc.sync.dma_start(out=waht[:], in_=waht_ext[:])
            wcat = pp.tile([128, 4, 4, 128], bf)
            nc.sync.dma_start(out=wcat[:], in_=wcat_ext[:])
            wihh = pp.tile([128, 4, 2 * 3 * H], bf)
            nc.sync.dma_start(out=wihh[:], in_=wihh_ext[:])
            sel4 = pp.tile([128, 4], bf)
            nc.sync.dma_start(out=sel4[:], in_=sel4_ext[:])
            ones = pp.tile([128, 2], bf)
            nc.sync.dma_start(out=ones[:], in_=ones_ext[:])
            onesr = pp.tile([1, 128], bf)
            nc.sync.dma_start(out=onesr[:], in_=onesr_ext[:])
            id16 = pp.tile([16, 16], bf)
            nc.sync.dma_start(out=id16[:], in_=id16_ext[:])
            h0T = pp.tile([128, 4, BL], bf)
            nc.sync.dma_start(out=h0T[:], in_=h0T_ext[:])
            h_n0 = pp.tile([BL, H], f32)
            nc.sync.dma_start(out=h_n0[:], in_=h0_ext[:])

            h_n_prev = h_n0
            hT_prev = h0T  # AP [128, 4, BL]

            cwm = CH * BL
            for c in range(NCH):
                # ---- chunk precompute: DA/DC for steps [c*CH, c*CH+cs) ----
                cs = min(CH, T - c * CH)
                cw = cs * BL
                c0 = c * CH * BL
                dect = dp.tile([128, 5, cwm], bf, tag="dect")
                nc.gpsimd.dma_start(
                    out=dect[:, :, 0:cw],
                    in_=decT_ext[:, :, c0:c0 + cw].rearrange("vt vp c -> vp vt c"))
                dat = cp.tile([128, 8, cwm], bf, tag="dat")
                dct = cp.tile([128, 4, cwm], bf, tag="dct")
                for st in range(8):
                    # stream WaVT k-tiles for this s-tile: [5][128,128]
                    pda = pmix.tile([128, 512], f32, tag="mix")
                    for kt in range(5):
                        wv = wsp.tile([128, 128], bf, tag="wav")
                        nc.gpsimd.dma_start(out=wv[:], in_=wavt_ext[kt, :, st, :])
                        nc.tensor.matmul(pda[:, 0:cw], wv[:], dect[:, kt, 0:cw],
                                         start=(kt == 0), stop=(kt == 4))
                    nc.scalar.copy(out=dat[:, st, 0:cw], in_=pda[:, 0:cw])
                for ot in range(4):
                    pdc = pmix.tile([128, 512], f32, tag="mix")
                    for kt in range(5):
                        wv = wsp.tile([128, 128], bf, tag="wcv")
                        nc.gpsimd.dma_start(out=wv[:], in_=wcvt_ext[kt, :, ot, :])
                        nc.tensor.matmul(pdc[:, 0:cw], wv[:], dect[:, kt, 0:cw],
                                         start=(kt == 0), stop=(kt == 4))
                    nc.scalar.copy(out=dct[:, ot, 0:cw], in_=pdc[:, 0:cw])

                hck = cp.tile([128, 4, cwm], bf, tag="hck")

                for j in range(cs):
                    t = c * CH + j
                    jb = j * BL
                    # 1. logitsT [s, b]: 8 s-tiles x 4 k
                    plg = pmix.tile([128, 128], f32, tag="mix")
                    for st in range(8):
                        for kt in range(4):
                            nc.tensor.matmul(
                                plg[:, st * BL:(st + 1) * BL],
                                waht[:, kt, st, :], hT_prev[:, kt, :],
                                start=(kt == 0), stop=(kt == 3))
                    # 2. P = exp(logits + DA_t)  -> bf16 [128, 8, 16]
                    tl = tp.tile([128, 8, BL], f32, tag="t128")
                    nc.vector.tensor_add(
                        tl[:], plg[:].rearrange("p (st b) -> p st b", st=8),
                        dat[:, :, jb:jb + BL])
                    pt = sp.tile([128, 8, BL], bf, tag="pt")
                    nc.scalar.activation(
                        out=pt[:], in_=tl[:],
                        func=mybir.ActivationFunctionType.Exp)
                    # 3. denom [16, 1]
                    pdn = pmix.tile([BL, 2], f32, tag="mix")
                    for st in range(8):
                        nc.tensor.matmul(pdn[:, 0:1], pt[:, st, :], ones[:, 0:1],
                                         start=(st == 0), stop=(st == 7))
                    # 4. recip
                    rec = tp.tile([BL, 1], f32, tag="rec")
                    nc.vector.reciprocal(rec[:], pdn[:, 0:1])
                    recb = tp.tile([BL, 1], bf, tag="recb")
                    nc.vector.tensor_copy(recb[:], rec[:])
                    # 5. recipT via PE transpose -> [1, 16]
                    prt = pmix.tile([1, 16], bf, tag="mix")
                    nc.tensor.transpose(prt[:], recb[:], id16[:])
                    rtb = tp.tile([1, 16], bf, tag="rtb")
                    nc.scalar.copy(out=rtb[:], in_=prt[:])
                    # 6. recip_row [128, 16] via K=1 matmul with ones
                    prr = pmix.tile([128, 16], f32, tag="mix")
                    nc.tensor.matmul(prr[:], onesr[:], rtb[:], start=True, stop=True)
                    rr = tp.tile([128, 16], bf, tag="rr")
                    nc.vector.tensor_copy(rr[:], prr[:])
                    # 7. PT_scaled = PT * recip_row  (broadcast over st)
                    pts = sp.tile([128, 8, BL], bf, tag="pts")
                    rrb = bass.AP(tensor=rr.tensor, offset=rr.offset,
                                  ap=[rr.ap[0], [0, 8], rr.ap[1]])
                    nc.vector.tensor_mul(pts[:], pt[:], rrb)
                    # 8/9. attention col-tiled + DTD per pack
                    patT = pmix.tile([128, 4, BL], f32, tag="mix")
                    for p in range(4):
                        pat = patt.tile([128, A], f32, tag="att")
                        if t < 2:
                            # init PSUM slot: DTD contracts all 128 rows with a
                            # 0/1 selector; unwritten rows must be finite
                            nc.vector.memset(pat[:, :], 0.0)
                        for st in range(8):
                            for g in range(4):
                                b = 4 * p + g
                                nc.tensor.matmul(
                                    pat[32 * g:32 * g + 1, :],
                                    pts[:, st, b:b + 1],
                                    E_sb[:, b, st, :],
                                    start=(st == 0), stop=(st == 7),
                                    tile_position=(0, 32 * g))
                        asp = sp.tile([128, A], bf, tag="asp")
                        if p % 2 == 0:
                            nc.vector.tensor_copy(asp[:, :], pat[:, :])
                        else:
                            nc.scalar.copy(out=asp[:, :], in_=pat[:, :])
                        for cc in range(4):
                            nc.tensor.matmul(
                                patT[:, cc, 4 * p:4 * p + 4],
                                asp[:, 128 * cc:128 * (cc + 1)],
                                sel4[:, :], start=True, stop=True)
                    attT = sp.tile([128, 4, BL], bf, tag="attT")
                    nc.scalar.copy(out=attT[:], in_=patT[:])
                    # 10. gru_preT = WcAT @ attT ; add DC, relu -> bf16
                    pgp = pmix.tile([128, 4, BL], f32, tag="mix")
                    for ot in range(4):
                        for at in range(4):
                            nc.tensor.matmul(pgp[:, ot, :], wcat[:, at, ot, :],
                                             attT[:, at, :],
                                             start=(at == 0), stop=(at == 3))
                    tg = tp.tile([128, 4, BL], f32, tag="t64")
                    nc.vector.tensor_add(tg[:], pgp[:], dct[:, :, jb:jb + BL])
                    gruT = sp.tile([128, 4, BL], bf, tag="gruT")
                    nc.scalar.activation(out=gruT[:], in_=tg[:],
                                         func=mybir.ActivationFunctionType.Relu)
                    # 11. gates: rz accum (gi+gh), n separate
                    przt = prz.tile([BL, 2, H], f32, tag="rz")
                    for half, o0 in ((0, 0), (1, H)):
                        mm_i = 0
                        for (src, woff) in ((gruT, 0), (hT_prev, 3 * H)):
                            for kt in range(4):
                                nc.tensor.matmul(
                                    przt[:, half, :], src[:, kt, :],
                                    wihh[:, kt, woff + o0:woff + o0 + H],
                                    start=(mm_i == 0), stop=(mm_i == 7))
                                mm_i += 1
                    pngi = pn.tile([BL, H], f32, tag="n")
                    for kt in range(4):
                        nc.tensor.matmul(pngi[:], gruT[:, kt, :],
                                         wihh[:, kt, 2 * H:3 * H],
                                         start=(kt == 0), stop=(kt == 3))
                    pngh = pn.tile([BL, H], f32, tag="n")
                    for kt in range(4):
                        nc.tensor.matmul(pngh[:], hT_prev[:, kt, :],
                                         wihh[:, kt, 5 * H:6 * H],
                                         start=(kt == 0), stop=(kt == 3))
                    # 12. r, z
                    rsb = tp.tile([BL, H], bf, tag="rsb")
                    nc.scalar.activation(out=rsb[:], in_=przt[:, 0, :],
                                         func=mybir.ActivationFunctionType.Sigmoid)
                    nc.scalar.activation(out=przt[:, 1, :], in_=przt[:, 1, :],
                                         func=mybir.ActivationFunctionType.Sigmoid)
                    # 13. n = tanh(gi_n + r * gh_n)
                    t1 = tp.tile([BL, H], f32, tag="tn")
                    nc.vector.tensor_mul(t1[:], rsb[:], pngh[:])
                    t2 = tp.tile([BL, H], f32, tag="tn")
                    nc.vector.tensor_add(t2[:], t1[:], pngi[:])
                    nsb = npo.tile([BL, H], f32, tag="nsb")
                    nc.scalar.activation(out=nsb[:], in_=t2[:],
                                         func=mybir.ActivationFunctionType.Tanh)
                    # 14. h' = n + z * (h - n)
                    d1 = tp.tile([BL, H], f32, tag="tn")
                    nc.vector.tensor_sub(d1[:], h_n_prev[:], nsb[:])
                    d2 = tp.tile([BL, H], f32, tag="tn")
                    nc.vector.tensor_mul(d2[:], przt[:, 1, :], d1[:])
                    h_n = sp.tile([BL, H], f32, tag="hn")
                    nc.vector.tensor_add(h_n[:], nsb[:], d2[:])
                    # 15. cast + transpose h -> hT (into h chunk)
                    hb = tp.tile([BL, H], bf, tag="rsb")
                    nc.vector.tensor_copy(hb[:], h_n[:])
                    phT = pmix.tile([128, 4, BL], bf, tag="mix")
                    for kt in range(4):
                        nc.tensor.transpose(phT[:, kt, :], hb[:, 128 * kt:128 * (kt + 1)],
                                            id16[:])
                    nc.scalar.copy(out=hck[:, :, jb:jb + BL], in_=phT[:])
                    hT_prev = hck[:, :, jb:jb + BL]
                    h_n_prev = h_n
                    if t == T - 1:
                        nc.sync.dma_start(out=hf_ext[:], in_=h_n[:])
                # DMA h chunk to stage
                nc.gpsimd.dma_start(out=hstage[:, :, c0:c0 + cw], in_=hck[:, :, 0:cw])

            # ---- output projection: out = h_all @ Wo.T ----
            wot = pp.tile([128, 4, V], bf)
            nc.sync.dma_start(out=wot[:], in_=wot_ext[:])
            nm = (TB + 127) // 128  # 19
            for m in range(nm):
                m0 = m * 128
                mw = min(128, TB - m0)
                hw = sp.tile([128, 4, 128], bf, tag="hw")
                nc.sync.dma_start(out=hw[:, :, 0:mw], in_=hstage[:, :, m0:m0 + mw])
                po = patt.tile([128, A], f32, tag="att")
                po2 = pn.tile([128, V - A], f32, tag="n")
                for kt in range(4):
                    nc.tensor.matmul(po[0:mw, :], hw[:, kt, 0:mw], wot[:, kt, 0:A],
                                     start=(kt == 0), stop=(kt == 3))
                for kt in range(4):
                    nc.tensor.matmul(po2[0:mw, :], hw[:, kt, 0:mw], wot[:, kt, A:V],
                                     start=(kt == 0), stop=(kt == 3))
                osb = sp.tile([128, V], f32, tag="osb")
                nc.vector.tensor_copy(osb[0:mw, 0:A], po[0:mw, :])
                nc.scalar.copy(out=osb[0:mw, A:V], in_=po2[0:mw, :])
                for ti in range(mw // BL):
                    t = (m0 + ti * BL) // BL
                    nc.sync.dma_start(out=out_ext[:, t, :],
                                      in_=osb[ti * BL:(ti + 1) * BL, :])

    nc.compile()
    return nc


def kernel(encoder_outputs, hidden, target_seq, max_length,
           Wa, ba, Wc, bc, W_ih, W_hh, b_ih, b_hh, Wo, bo):
    from concourse.bass_utils import run_bass_kernel_spmd

    encoder_outputs = np.asarray(encoder_outputs, np.float32)
    hidden = np.asarray(hidden, np.float32)
    target_seq = np.asarray(target_seq, np.float32)
    Wa = np.asarray(Wa, np.float32); Wc = np.asarray(Wc, np.float32)
    W_ih = np.asarray(W_ih, np.float32); W_hh = np.asarray(W_hh, np.float32)
    Wo = np.asarray(Wo, np.float32)
    assert int(max_length) == T, (max_length, T)
    for bias in (ba, bc, b_ih, b_hh, bo):
        assert not np.any(np.asarray(bias)), "nonzero biases unsupported"

    if "nc" not in _CACHE:
        _CACHE["nc"] = _build()
    nc = _CACHE["nc"]

    # host-side weight prep (shared across cores)
    WaV, WaH = Wa[:, :V], Wa[:, V:]
    WcV, WcA = Wc[:, :V], Wc[:, V:]
    waht = np.ascontiguousarray(
        WaH.T.reshape(4, 128, 8, 128).transpose(1, 0, 2, 3)).astype(bf16)
    wavt_f = np.zeros((VP, S), np.float32); wavt_f[:V] = WaV.T
    wavt = np.ascontiguousarray(wavt_f.reshape(5, 128, 8, 128)).astype(bf16)
    wcat = np.ascontiguousarray(
        WcA.T.reshape(4, 128, 4, 128).transpose(1, 0, 2, 3)).astype(bf16)
    wcvt_f = np.zeros((VP, H), np.float32); wcvt_f[:V] = WcV.T
    wcvt = np.ascontiguousarray(wcvt_f.reshape(5, 128, 4, 128)).astype(bf16)
    wihh_f = np.concatenate([W_ih.T, W_hh.T], axis=1)  # [512, 3072]
    wihh = np.ascontiguousarray(
        wihh_f.reshape(4, 128, 6 * H).transpose(1, 0, 2)).astype(bf16)
    wot = np.ascontiguousarray(
        Wo.T.reshape(4, 128, V).transpose(1, 0, 2)).astype(bf16)
    sel4 = np.zeros((128, 4), np.float32)
    for g in range(4):
        sel4[32 * g, g] = 1.0
    sel4 = sel4.astype(bf16)
    ones = np.zeros((128, 2), np.float32); ones[:, 0] = 1.0
    ones = ones.astype(bf16)
    onesr = np.ones((1, 128), np.float32).astype(bf16)
    id16 = np.eye(16, dtype=np.float32).astype(bf16)

    in_maps = []
    for c in range(NC):
        bs = slice(c * BL, (c + 1) * BL)
        E = encoder_outputs[bs]                       # [16, 1024, 512]
        e_arr = np.ascontiguousarray(E.reshape(BL, 8, 128, A)).astype(bf16)
        h0 = hidden[-1][bs]                           # [16, 512]
        h0T = np.ascontiguousarray(
            h0.T.reshape(4, 128, BL).transpose(1, 0, 2)).astype(bf16)
        dec = np.concatenate(
            [np.zeros((BL, 1, V), np.float32), target_seq[bs, :T - 1]], axis=1)
        decT_f = np.zeros((VP, T, BL), np.float32)
        decT_f[:V] = dec.transpose(2, 1, 0)           # [v, t, b]
        dect = np.ascontiguousarray(
            decT_f.reshape(5, 128, TB)).astype(bf16)
        in_maps.append(dict(
            e=e_arr, h0t=h0T, h0=np.ascontiguousarray(h0),
            dect=dect, waht=waht, wavt=wavt, wcat=wcat, wcvt=wcvt,
            wihh=wihh, wot=wot, sel4=sel4, ones=ones, onesr=onesr, id16=id16))
    _CACHE["in_maps"] = in_maps

    res = run_bass_kernel_spmd(nc, in_maps, list(range(NC)))
    out_seq = np.concatenate([res.results[c]["out"] for c in range(NC)], axis=0)
    h_final = np.concatenate([res.results[c]["hf"] for c in range(NC)], axis=0)
    return out_seq.astype(np.float32), h_final[None].astype(np.float32)


def time_kernel(inputs, iters=10):
    """Repeated-execute timing through one persistent jitted executable."""
    import time as _time
    import jax
    from jax.sharding import Mesh, PartitionSpec
    from jax.experimental.shard_map import shard_map
    from concourse import bass2jax, mybir
    bass2jax.install_neuronx_cc_hook()

    kernel(**inputs)  # ensure _CACHE["nc"]; also warms neff path
    nc = _CACHE["nc"]
    in_maps = _CACHE["in_maps"]

    in_names, out_names, out_avals, zero_outs = [], [], [], []
    partition_name = nc.partition_id_tensor.name if nc.partition_id_tensor else None
    for alloc in nc.m.functions[0].allocations:
        if not isinstance(alloc, mybir.MemoryLocationSet):
            continue
        name = alloc.memorylocations[0].name
        if alloc.kind == "ExternalInput":
            if name != partition_name:
                in_names.append(name)
        elif alloc.kind == "ExternalOutput":
            shape = tuple(alloc.tensor_shape)
            dtype = mybir.dt.np(alloc.dtype)
            out_names.append(name)
            out_avals.append(jax.core.ShapedArray(shape, dtype))
            zero_outs.append(np.zeros(shape, dtype))
    n_params = len(in_names)
    all_names = list(in_names) + list(out_names)
    if partition_name is not None:
        all_names.append(partition_name)

    def _body(*args):
        operands = list(args)
        if partition_name is not None:
            operands.append(bass2jax.partition_id_tensor())
        outs = bass2jax._bass_exec_p.bind(
            *operands, out_avals=tuple(out_avals), in_names=tuple(all_names),
            out_names=tuple(out_names), lowering_input_output_aliases=(),
            sim_require_finite=True, sim_require_nnan=True, nc=nc)
        return tuple(outs)

    devices = jax.devices()[:NC]
    mesh = Mesh(np.asarray(devices), ("core",))
    n_outs = len(out_names)
    sharded = jax.jit(shard_map(
        _body, mesh=mesh,
        in_specs=(PartitionSpec("core"),) * (n_params + n_outs),
        out_specs=(PartitionSpec("core"),) * n_outs, check_rep=False),
        keep_unused=True)
    concat_in = [np.concatenate([np.asarray(in_maps[c][n]) for c in range(NC)], axis=0)
                 for n in in_names]
    concat_zeros = [np.zeros((NC * z.shape[0], *z.shape[1:]), z.dtype)
                    for z in zero_outs]
    dev_in = [jax.device_put(a) for a in concat_in + concat_zeros]
    times = []
    for _ in range(iters):
        t0 = _time.time()
        outs = sharded(*dev_in)
        jax.block_until_ready(outs)
        times.append(_time.time() - t0)
    return times


# revision 40
# speedup vs baseline: 4.7334x; 4.7334x over previous
"""AttentionGRUDecoder Trainium2 kernel: 8-core batch-parallel (16 batch/core).

Strategy:
  - Shard B=128 across 8 cores (16 each); replicate weights. No collectives.
  - encoder_outputs resident in SBUF as bf16 (16 MB/core); streamed through PE
    each step via column-tiled (tile_position) matmuls, 4 concurrent streams.
  - dec_in-dependent matmuls (dec @ WaV.T, dec @ WcV.T) hoisted out of the
    recurrence, computed per 10-step chunk, overlapped with the loop.
  - Output projection (h @ Wo.T) batched after the loop over all 150 steps.
  - h kept in fp32 between steps; bf16 only feeds matmuls.
"""
import sys
sys.path.insert(0, '/opt/trn_rl_repo')
import numpy as np
import ml_dtypes

import os
B, T, H, A, V = 128, 150, 512, 512, 600
T = int(os.environ.get("KERNEL_T", T))
S = 2 * A            # 1024 attention positions
NC = 8               # cores
BL = B // NC         # 16 local batch
VP = 640             # V padded to 5*128
CH = min(8, T)       # steps per precompute chunk
NCH = (T + CH - 1) // CH   # chunks (last may be ragged)
TB = T * BL          # (t, b) columns

bf16 = ml_dtypes.bfloat16

_CACHE = {}


def _build():
    import concourse.bass as bass
    import concourse.mybir as mybir
    import concourse.tile as tile
    from concourse import bacc

    f32 = mybir.dt.float32
    bf = mybir.dt.bfloat16

    nc = bacc.Bacc("TRN2", target_bir_lowering=False, debug=False, num_devices=NC)
    dram = lambda n, s, d, k: nc.dram_tensor(n, s, d, kind=k).ap()

    # inputs (host-prepped layouts)
    E_ext = dram("e", [BL, 8, 128, A], bf, "ExternalInput")          # [b, st, sp, a]
    h0T_ext = dram("h0t", [128, 4, BL], bf, "ExternalInput")         # [kp, kt, b]
    h0_ext = dram("h0", [BL, H], f32, "ExternalInput")
    decT_ext = dram("dect", [5, 128, TB], bf, "ExternalInput")       # [vt, vp, (t,b)]
    waht_ext = dram("waht", [128, 4, 8, 128], bf, "ExternalInput")   # [kp, kt, st, ss]
    wavt_ext = dram("wavt", [5, 128, 8, 128], bf, "ExternalInput")   # [vt, vp, st, ss]
    wcat_ext = dram("wcat", [128, 4, 4, 128], bf, "ExternalInput")   # [ap, at, ot, os]
    wcvt_ext = dram("wcvt", [5, 128, 4, 128], bf, "ExternalInput")
    wihh_ext = dram("wihh", [128, 4, 2 * 3 * H], bf, "ExternalInput")  # [kp, kt, 3072]
    wot_ext = dram("wot", [128, 4, V], bf, "ExternalInput")          # [kp, kt, o]
    sel4_ext = dram("sel4", [128, 4], bf, "ExternalInput")
    ones_ext = dram("ones", [128, 2], bf, "ExternalInput")           # col 0: ones
    onesr_ext = dram("onesr", [1, 128], bf, "ExternalInput")
    id16_ext = dram("id16", [16, 16], bf, "ExternalInput")

    out_ext = dram("out", [BL, T, V], f32, "ExternalOutput")
    hf_ext = dram("hf", [BL, H], f32, "ExternalOutput")
    hstage = dram("hstage", [128, 4, TB], bf, "Internal")

    with tile.TileContext(nc) as tc:
        with tc.tile_pool(name="persist", bufs=1) as pp, \
             tc.tile_pool(name="wstream", bufs=2) as wsp, \
             tc.tile_pool(name="chunk", bufs=2) as cp, \
             tc.tile_pool(name="dpool", bufs=1) as dp, \
             tc.tile_pool(name="npool", bufs=1) as npo, \
             tc.tile_pool(name="step", bufs=2) as sp, \
             tc.tile_pool(name="tmp", bufs=2) as tp, \
             tc.tile_pool(name="pmix", bufs=2, space="PSUM") as pmix, \
             tc.tile_pool(name="patt", bufs=1, space="PSUM") as patt, \
             tc.tile_pool(name="prz", bufs=1, space="PSUM") as prz:

            # ---- persistent loads ----
            E_sb = pp.tile([128, BL, 8, A], bf)
            for b in range(BL):
                nc.sync.dma_start(
                    out=E_sb[:, b, :, :],
                    in_=E_ext[b].rearrange("st sp a -> sp st a"))
            waht = pp.tile([128, 4, 8, 128], bf)
            nc.sync.dma_start(out=waht[:], in_=waht_ext[:])
            wcat = pp.tile([128, 4, 4, 128], bf)
            nc.sync.dma_start(out=wcat[:], in_=wcat_ext[:])
            wihh = pp.tile([128, 4, 2 * 3 * H], bf)
            nc.sync.dma_start(out=wihh[:], in_=wihh_ext[:])
            sel4 = pp.tile([128, 4], bf)
            nc.sync.dma_start(out=sel4[:], in_=sel4_ext[:])
            ones = pp.tile([128, 2], bf)
            nc.sync.dma_start(out=ones[:], in_=ones_ext[:])
            onesr = pp.tile([1, 128], bf)
            nc.sync.dma_start(out=onesr[:], in_=onesr_ext[:])
            id16 = pp.tile([16, 16], bf)
            nc.sync.dma_start(out=id16[:], in_=id16_ext[:])
            h0T = pp.tile([128, 4, BL], bf)
            nc.sync.dma_start(out=h0T[:], in_=h0T_ext[:])
            h_n0 = pp.tile([BL, H], f32)
            nc.sync.dma_start(out=h_n0[:], in_=h0_ext[:])

            h_n_prev = h_n0
            hT_prev = h0T  # AP [128, 4, BL]

            # persistent attention PSUM accumulators (memset once: the dense
            # DTD selector contraction requires all 128 rows finite)
            pat_ps = [patt.tile([128, A], f32, tag=f"att{i}", name=f"pat{i}")
                      for i in range(2)]
            for _pp in pat_ps:
                nc.vector.memset(_pp[:, :], 0.0)

            cwm = CH * BL
            for c in range(NCH):
                # ---- chunk precompute: DA/DC for steps [c*CH, c*CH+cs) ----
                cs = min(CH, T - c * CH)
                cw = cs * BL
                c0 = c * CH * BL
                dect = dp.tile([128, 5, cwm], bf, tag="dect")
                nc.gpsimd.dma_start(
                    out=dect[:, :, 0:cw],
                    in_=decT_ext[:, :, c0:c0 + cw].rearrange("vt vp c -> vp vt c"))
                dat = cp.tile([128, 8, cwm], bf, tag="dat")
                dct = cp.tile([128, 4, cwm], bf, tag="dct")
                for st in range(8):
                    # stream WaVT k-tiles for this s-tile: [5][128,128]
                    pda = pmix.tile([128, 512], f32, tag="mix")
                    for kt in range(5):
                        wv = wsp.tile([128, 128], bf, tag="wav")
                        nc.gpsimd.dma_start(out=wv[:], in_=wavt_ext[kt, :, st, :])
                        nc.tensor.matmul(pda[:, 0:cw], wv[:], dect[:, kt, 0:cw],
                                         start=(kt == 0), stop=(kt == 4))
                    nc.scalar.copy(out=dat[:, st, 0:cw], in_=pda[:, 0:cw])
                for ot in range(4):
                    pdc = pmix.tile([128, 512], f32, tag="mix")
                    for kt in range(5):
                        wv = wsp.tile([128, 128], bf, tag="wcv")
                        nc.gpsimd.dma_start(out=wv[:], in_=wcvt_ext[kt, :, ot, :])
                        nc.tensor.matmul(pdc[:, 0:cw], wv[:], dect[:, kt, 0:cw],
                                         start=(kt == 0), stop=(kt == 4))
                    nc.scalar.copy(out=dct[:, ot, 0:cw], in_=pdc[:, 0:cw])

                hck = cp.tile([128, 4, cwm], bf, tag="hck")

                for j in range(cs):
                    t = c * CH + j
                    jb = j * BL
                    # 1. logitsT [s, b]: 8 s-tiles x 4 k
                    plg = pmix.tile([128, 128], f32, tag="mix")
                    for st in range(8):
                        for kt in range(4):
                            nc.tensor.matmul(
                                plg[:, st * BL:(st + 1) * BL],
                                waht[:, kt, st, :], hT_prev[:, kt, :],
                                start=(kt == 0), stop=(kt == 3))
                    # 2. P = exp(logits + DA_t)  -> bf16 [128, 8, 16]
                    tl = tp.tile([128, 8, BL], f32, tag="t128")
                    nc.vector.tensor_add(
                        tl[:], plg[:].rearrange("p (st b) -> p st b", st=8),
                        dat[:, :, jb:jb + BL])
                    pt = sp.tile([128, 8, BL], bf, tag="pt")
                    nc.scalar.activation(
                        out=pt[:], in_=tl[:],
                        func=mybir.ActivationFunctionType.Exp)
                    # 3. denom [16, 1]
                    pdn = pmix.tile([BL, 2], f32, tag="mix")
                    for st in range(8):
                        nc.tensor.matmul(pdn[:, 0:1], pt[:, st, :], ones[:, 0:1],
                                         start=(st == 0), stop=(st == 7))
                    # 4. recip
                    rec = tp.tile([BL, 1], f32, tag="rec")
                    nc.vector.reciprocal(rec[:], pdn[:, 0:1])
                    recb = tp.tile([BL, 1], bf, tag="recb")
                    nc.vector.tensor_copy(recb[:], rec[:])
                    # 5. recipT via PE transpose -> [1, 16]
                    prt = pmix.tile([1, 16], bf, tag="mix")
                    nc.tensor.transpose(prt[:], recb[:], id16[:])
                    rtb = tp.tile([1, 16], bf, tag="rtb")
                    nc.scalar.copy(out=rtb[:], in_=prt[:])
                    # 6. recip_row [128, 16] via K=1 matmul with ones
                    prr = pmix.tile([128, 16], f32, tag="mix")
                    nc.tensor.matmul(prr[:], onesr[:], rtb[:], start=True, stop=True)
                    rr = tp.tile([128, 16], bf, tag="rr")
                    nc.vector.tensor_copy(rr[:], prr[:])
                    # 7. PT_scaled = PT * recip_row  (broadcast over st)
                    pts = sp.tile([128, 8, BL], bf, tag="pts")
                    rrb = bass.AP(tensor=rr.tensor, offset=rr.offset,
                                  ap=[rr.ap[0], [0, 8], rr.ap[1]])
                    nc.vector.tensor_mul(pts[:], pt[:], rrb)
                    # 8/9. attention col-tiled + DTD per pack
                    patT = pmix.tile([128, 4, BL], f32, tag="mix")
                    for p in range(4):
                        pat = pat_ps[p % 2]
                        for st in range(8):
                            for g in range(4):
                                b = 4 * p + g
                                nc.tensor.matmul(
                                    pat[32 * g:32 * g + 1, :],
                                    pts[:, st, b:b + 1],
                                    E_sb[:, b, st, :],
                                    start=(st == 0), stop=(st == 7),
                                    tile_position=(0, 32 * g))
                        asp = sp.tile([128, A], bf, tag="asp")
                        if p % 2 == 0:
                            nc.vector.tensor_copy(asp[:, :], pat[:, :])
                        else:
                            nc.scalar.copy(out=asp[:, :], in_=pat[:, :])
                        for cc in range(4):
                            nc.tensor.matmul(
                                patT[:, cc, 4 * p:4 * p + 4],
                                asp[:, 128 * cc:128 * (cc + 1)],
                                sel4[:, :], start=True, stop=True)
                    attT = sp.tile([128, 4, BL], bf, tag="attT")
                    nc.scalar.copy(out=attT[:], in_=patT[:])
                    # 10. gru_preT = WcAT @ attT ; add DC, relu -> bf16
                    pgp = pmix.tile([128, 4, BL], f32, tag="mix")
                    for ot in range(4):
                        for at in range(4):
                            nc.tensor.matmul(pgp[:, ot, :], wcat[:, at, ot, :],
                                             attT[:, at, :],
                                             start=(at == 0), stop=(at == 3))
                    tg = tp.tile([128, 4, BL], f32, tag="t64")
                    nc.vector.tensor_add(tg[:], pgp[:], dct[:, :, jb:jb + BL])
                    gruT = sp.tile([128, 4, BL], bf, tag="gruT")
                    nc.scalar.activation(out=gruT[:], in_=tg[:],
                                         func=mybir.ActivationFunctionType.Relu)
                    # 11. gates, col-tiled: 4 concurrent chains at col groups
                    #     r@rows0-15, z@32-47, ngi@64-79, ngh@96-111 (1 bank)
                    pg4 = prz.tile([128, 4, H], f32, tag="rz")
                    pg = [pg4[32 * g:32 * g + BL, g, :] for g in range(4)]
                    chains = (
                        (0, ((gruT, 0), (hT_prev, 3 * H))),   # r
                        (1, ((gruT, H), (hT_prev, 4 * H))),   # z
                        (2, ((gruT, 2 * H),)),                # n-gi
                        (3, ((hT_prev, 5 * H),)),             # n-gh
                    )
                    mms = {g: [(src, woff, kt) for (src, woff) in parts
                               for kt in range(4)] for g, parts in chains}
                    for i in range(8):
                        for g, _ in chains:
                            lst = mms[g]
                            if i < len(lst):
                                src, woff, kt = lst[i]
                                nc.tensor.matmul(
                                    pg[g], src[:, kt, :],
                                    wihh[:, kt, woff:woff + H],
                                    start=(i == 0), stop=(i == len(lst) - 1),
                                    tile_position=(0, 32 * g))
                    # 12. r, z
                    rsb = tp.tile([BL, H], bf, tag="rsb")
                    nc.scalar.activation(out=rsb[:], in_=pg[0],
                                         func=mybir.ActivationFunctionType.Sigmoid)
                    nc.scalar.activation(out=pg[1], in_=pg[1],
                                         func=mybir.ActivationFunctionType.Sigmoid)
                    # 13. n = tanh(gi_n + r * gh_n)
                    t1 = tp.tile([BL, H], f32, tag="tn")
                    nc.vector.tensor_mul(t1[:], rsb[:], pg[3])
                    t2 = tp.tile([BL, H], f32, tag="tn")
                    nc.vector.tensor_add(t2[:], t1[:], pg[2])
                    nsb = npo.tile([BL, H], f32, tag="nsb")
                    nc.scalar.activation(out=nsb[:], in_=t2[:],
                                         func=mybir.ActivationFunctionType.Tanh)
                    # 14. h' = n + z * (h - n)
                    d1 = tp.tile([BL, H], f32, tag="tn")
                    nc.vector.tensor_sub(d1[:], h_n_prev[:], nsb[:])
                    d2 = tp.tile([BL, H], f32, tag="tn")
                    nc.vector.tensor_mul(d2[:], pg[1], d1[:])
                    h_n = sp.tile([BL, H], f32, tag="hn")
                    nc.vector.tensor_add(h_n[:], nsb[:], d2[:])
                    # 15. cast + transpose h -> hT (into h chunk)
                    hb = tp.tile([BL, H], bf, tag="rsb")
                    nc.vector.tensor_copy(hb[:], h_n[:])
                    phT = pmix.tile([128, 4, BL], bf, tag="mix")
                    for kt in range(4):
                        nc.tensor.transpose(phT[:, kt, :], hb[:, 128 * kt:128 * (kt + 1)],
                                            id16[:])
                    nc.scalar.copy(out=hck[:, :, jb:jb + BL], in_=phT[:])
                    hT_prev = hck[:, :, jb:jb + BL]
                    h_n_prev = h_n
                    if t == T - 1:
                        nc.sync.dma_start(out=hf_ext[:], in_=h_n[:])
                # DMA h chunk to stage
                nc.gpsimd.dma_start(out=hstage[:, :, c0:c0 + cw], in_=hck[:, :, 0:cw])

            # ---- output projection: out = h_all @ Wo.T ----
            wot = pp.tile([128, 4, V], bf)
            nc.sync.dma_start(out=wot[:], in_=wot_ext[:])
            nm = (TB + 127) // 128  # 19
            for m in range(nm):
                m0 = m * 128
                mw = min(128, TB - m0)
                hw = sp.tile([128, 4, 128], bf, tag="hw")
                nc.sync.dma_start(out=hw[:, :, 0:mw], in_=hstage[:, :, m0:m0 + mw])
                po = pat_ps[m % 2]
                po2 = pmix.tile([128, V - A], f32, tag="mix")
                for kt in range(4):
                    nc.tensor.matmul(po[0:mw, :], hw[:, kt, 0:mw], wot[:, kt, 0:A],
                                     start=(kt == 0), stop=(kt == 3))
                for kt in range(4):
                    nc.tensor.matmul(po2[0:mw, :], hw[:, kt, 0:mw], wot[:, kt, A:V],
                                     start=(kt == 0), stop=(kt == 3))
                osb = sp.tile([128, V], f32, tag="osb")
                nc.vector.tensor_copy(osb[0:mw, 0:A], po[0:mw, :])
                nc.scalar.copy(out=osb[0:mw, A:V], in_=po2[0:mw, :])
                for ti in range(mw // BL):
                    t = (m0 + ti * BL) // BL
                    nc.sync.dma_start(out=out_ext[:, t, :],
                                      in_=osb[ti * BL:(ti + 1) * BL, :])

    nc.compile()
    return nc


def kernel(encoder_outputs, hidden, target_seq, max_length,
           Wa, ba, Wc, bc, W_ih, W_hh, b_ih, b_hh, Wo, bo):
    from concourse.bass_utils import run_bass_kernel_spmd

    encoder_outputs = np.asarray(encoder_outputs, np.float32)
    hidden = np.asarray(hidden, np.float32)
    target_seq = np.asarray(target_seq, np.float32)
    Wa = np.asarray(Wa, np.float32); Wc = np.asarray(Wc, np.float32)
    W_ih = np.asarray(W_ih, np.float32); W_hh = np.asarray(W_hh, np.float32)
    Wo = np.asarray(Wo, np.float32)
    assert int(max_length) == T, (max_length, T)
    for bias in (ba, bc, b_ih, b_hh, bo):
        assert not np.any(np.asarray(bias)), "nonzero biases unsupported"

    if "nc" not in _CACHE:
        _CACHE["nc"] = _build()
    nc = _CACHE["nc"]

    # host-side weight prep (shared across cores)
    WaV, WaH = Wa[:, :V], Wa[:, V:]
    WcV, WcA = Wc[:, :V], Wc[:, V:]
    waht = np.ascontiguousarray(
        WaH.T.reshape(4, 128, 8, 128).transpose(1, 0, 2, 3)).astype(bf16)
    wavt_f = np.zeros((VP, S), np.float32); wavt_f[:V] = WaV.T
    wavt = np.ascontiguousarray(wavt_f.reshape(5, 128, 8, 128)).astype(bf16)
    wcat = np.ascontiguousarray(
        WcA.T.reshape(4, 128, 4, 128).transpose(1, 0, 2, 3)).astype(bf16)
    wcvt_f = np.zeros((VP, H), np.float32); wcvt_f[:V] = WcV.T
    wcvt = np.ascontiguousarray(wcvt_f.reshape(5, 128, 4, 128)).astype(bf16)
    wihh_f = np.concatenate([W_ih.T, W_hh.T], axis=1)  # [512, 3072]
    wihh = np.ascontiguousarray(
        wihh_f.reshape(4, 128, 6 * H).transpose(1, 0, 2)).astype(bf16)
    wot = np.ascontiguousarray(
        Wo.T.reshape(4, 128, V).transpose(1, 0, 2)).astype(bf16)
    sel4 = np.zeros((128, 4), np.float32)
    for g in range(4):
        sel4[32 * g, g] = 1.0
    sel4 = sel4.astype(bf16)
    ones = np.zeros((128, 2), np.float32); ones[:, 0] = 1.0
    ones = ones.astype(bf16)
    onesr = np.ones((1, 128), np.float32).astype(bf16)
    id16 = np.eye(16, dtype=np.float32).astype(bf16)

    in_maps = []
    for c in range(NC):
        bs = slice(c * BL, (c + 1) * BL)
        E = encoder_outputs[bs]                       # [16, 1024, 512]
        e_arr = np.ascontiguousarray(E.reshape(BL, 8, 128, A)).astype(bf16)
        h0 = hidden[-1][bs]                           # [16, 512]
        h0T = np.ascontiguousarray(
            h0.T.reshape(4, 128, BL).transpose(1, 0, 2)).astype(bf16)
        dec = np.concatenate(
            [np.zeros((BL, 1, V), np.float32), target_seq[bs, :T - 1]], axis=1)
        decT_f = np.zeros((VP, T, BL), np.float32)
        decT_f[:V] = dec.transpose(2, 1, 0)           # [v, t, b]
        dect = np.ascontiguousarray(
            decT_f.reshape(5, 128, TB)).astype(bf16)
        in_maps.append(dict(
            e=e_arr, h0t=h0T, h0=np.ascontiguousarray(h0),
            dect=dect, waht=waht, wavt=wavt, wcat=wcat, wcvt=wcvt,
            wihh=wihh, wot=wot, sel4=sel4, ones=ones, onesr=onesr, id16=id16))
    _CACHE["in_maps"] = in_maps

    res = run_bass_kernel_spmd(nc, in_maps, list(range(NC)))
    out_seq = np.concatenate([res.results[c]["out"] for c in range(NC)], axis=0)
    h_final = np.concatenate([res.results[c]["hf"] for c in range(NC)], axis=0)
    return out_seq.astype(np.float32), h_final[None].astype(np.float32)


def time_kernel(inputs, iters=10):
    """Repeated-execute timing through one persistent jitted executable."""
    kernel(**inputs)  # ensure _CACHE["nc"]; also warms neff path
    return _time_nc(_CACHE["nc"], _CACHE["in_maps"], iters)


def time_floor(iters=10):
    """Dispatch-overhead floor: trivial NEFF through the same path."""
    import concourse.mybir as mybir
    import concourse.tile as tile
    from concourse import bacc
    nc = bacc.Bacc("TRN2", target_bir_lowering=False, debug=False, num_devices=NC)
    x = nc.dram_tensor("x", [128, 512], mybir.dt.float32, kind="ExternalInput").ap()
    o = nc.dram_tensor("o", [128, 512], mybir.dt.float32, kind="ExternalOutput").ap()
    with tile.TileContext(nc) as tc:
        with tc.tile_pool(name="p", bufs=2) as pool:
            t = pool.tile([128, 512], mybir.dt.float32)
            nc.sync.dma_start(out=t[:], in_=x[:])
            t2 = pool.tile([128, 512], mybir.dt.float32)
            nc.scalar.mul(out=t2[:], in_=t[:], mul=2.0)
            nc.sync.dma_start(out=o[:], in_=t2[:])
    nc.compile()
    in_maps = [{"x": np.zeros((128, 512), np.float32)} for _ in range(NC)]
    return _time_nc(nc, in_maps, iters)


def _time_nc(nc, in_maps, iters):
    import time as _time
    import jax
    from jax.sharding import Mesh, PartitionSpec
    from jax.experimental.shard_map import shard_map
    from concourse import bass2jax, mybir
    bass2jax.install_neuronx_cc_hook()

    in_names, out_names, out_avals, zero_outs = [], [], [], []
    partition_name = nc.partition_id_tensor.name if nc.partition_id_tensor else None
    for alloc in nc.m.functions[0].allocations:
        if not isinstance(alloc, mybir.MemoryLocationSet):
            continue
        name = alloc.memorylocations[0].name
        if alloc.kind == "ExternalInput":
            if name != partition_name:
                in_names.append(name)
        elif alloc.kind == "ExternalOutput":
            shape = tuple(alloc.tensor_shape)
            dtype = mybir.dt.np(alloc.dtype)
            out_names.append(name)
            out_avals.append(jax.core.ShapedArray(shape, dtype))
            zero_outs.append(np.zeros(shape, dtype))
    n_params = len(in_names)
    all_names = list(in_names) + list(out_names)
    if partition_name is not None:
        all_names.append(partition_name)

    def _body(*args):
        operands = list(args)
        if partition_name is not None:
            operands.append(bass2jax.partition_id_tensor())
        outs = bass2jax._bass_exec_p.bind(
            *operands, out_avals=tuple(out_avals), in_names=tuple(all_names),
            out_names=tuple(out_names), lowering_input_output_aliases=(),
            sim_require_finite=True, sim_require_nnan=True, nc=nc)
        return tuple(outs)

    devices = jax.devices()[:NC]
    mesh = Mesh(np.asarray(devices), ("core",))
    n_outs = len(out_names)
    sharded = jax.jit(shard_map(
        _body, mesh=mesh,
        in_specs=(PartitionSpec("core"),) * (n_params + n_outs),
        out_specs=(PartitionSpec("core"),) * n_outs, check_rep=False),
        keep_unused=True)
    concat_in = [np.concatenate([np.asarray(in_maps[c][n]) for c in range(NC)], axis=0)
                 for n in in_names]
    concat_zeros = [np.zeros((NC * z.shape[0], *z.shape[1:]), z.dtype)
                    for z in zero_outs]
    dev_in = [jax.device_put(a) for a in concat_in + concat_zeros]
    times = []
    for _ in range(iters):
        t0 = _time.time()
        outs = sharded(*dev_in)
        jax.block_until_ready(outs)
        times.append(_time.time() - t0)
    return times
